# revision 4
# baseline (speedup 1.0000x reference)
"""DIEN-style interest kernel (GRU -> DIN attention -> AUGRU) for TRN2.

Sharding: pure data parallel, batch 1024 -> 8 cores x 128 rows.
Layout: B-layout recurrence (batch on partitions). Per step t:
  psumA[B,256] = x_t-stationary gi(rz) matmul + h-stationary gh(rz) matmul (PSUM accum)
  psumB[B,256] = [gh_n | gi_n] side by side
  r,s = sigmoid(psumA)   (z-block weights negated on host so s = 1-z for the GRU)
  n = tanh(r*gh_n + gi_n)
  h' = h + s*m_t*(n-h)   (GRU-E; mask column fused via scalar_tensor_tensor)
  h' = h + u*a_t*(n-h)   (AUGRU; attention column fused the same way)
  hT = PE-transpose(h') -> bf16 copy (doubles as transposed interests store)
Attention MLP + logits computed in 8-step chunks overlapped with GRU-E;
softmax in [B,T] layout with ACT Exp + accum_out.
"""

import os
import sys

sys.path.insert(0, "/opt/trn_rl_repo")

import ml_dtypes
import numpy as np

B_TOT, T, H = 1024, 200, 128
NCORES = 8
B = B_TOT // NCORES  # 128 rows per core
TC = 8               # time steps per attention chunk
NCH = T // TC        # 25 chunks
HID1, HID2 = 80, 40

_PROG = None
LAST_EXEC_NS = None


def _build_program():
    import concourse.mybir as mybir
    import concourse.tile as tile
    from concourse import bacc
    from concourse.masks import make_identity

    dt = mybir.dt
    f32, bf16 = dt.float32, dt.bfloat16
    AF = mybir.ActivationFunctionType
    OP = mybir.AluOpType

    nc = bacc.Bacc(None)

    # ---- DRAM parameters (host-prepared layouts) ----
    d_keysT = nc.declare_dram_parameter("keysT", [H, T * B], bf16, isOutput=False)
    d_intT = nc.declare_dram_parameter("qT", [H, B], bf16, isOutput=False)
    d_w = {}
    for g in ("e", "a"):
        d_w[f"{g}_whh_rz"] = nc.declare_dram_parameter(f"{g}_whh_rz", [H, 256], bf16, isOutput=False)
        d_w[f"{g}_whh_n"] = nc.declare_dram_parameter(f"{g}_whh_n", [H, 128], bf16, isOutput=False)
        d_w[f"{g}_wih_rz"] = nc.declare_dram_parameter(f"{g}_wih_rz", [H, 256], bf16, isOutput=False)
        d_w[f"{g}_wih_n"] = nc.declare_dram_parameter(f"{g}_wih_n", [H, 128], bf16, isOutput=False)
    d_w1k = nc.declare_dram_parameter("w1k", [H, HID1], bf16, isOutput=False)
    d_w1p = nc.declare_dram_parameter("w1p", [H, HID1], bf16, isOutput=False)
    d_w1q = nc.declare_dram_parameter("w1q", [H, HID1], bf16, isOutput=False)
    d_w2 = nc.declare_dram_parameter("w2", [HID1, HID2], bf16, isOutput=False)
    d_wf = nc.declare_dram_parameter("wf", [HID2, 1], bf16, isOutput=False)
    d_identrep = nc.declare_dram_parameter("identrep", [B, 512], bf16, isOutput=False)
    d_maskadd = nc.declare_dram_parameter("maskadd", [B, T], f32, isOutput=False)
    d_mmask = nc.declare_dram_parameter("mmask", [B, T], f32, isOutput=False)
    d_out = nc.declare_dram_parameter("out", [B, H], f32, isOutput=True)

    with tile.TileContext(nc) as tc:
        with (
            tc.tile_pool(name="consts", bufs=1) as consts,
            tc.tile_pool(name="keysp", bufs=NCH) as keysp,
            tc.tile_pool(name="intp", bufs=NCH) as intp,
            tc.tile_pool(name="qkp", bufs=3) as qkp,
            tc.tile_pool(name="state", bufs=4) as state,
            tc.tile_pool(name="gate", bufs=4) as gatep,
            tc.tile_pool(name="small", bufs=8) as small,
            tc.tile_pool(name="attn_sb", bufs=2) as attn_sb,
            tc.tile_pool(name="soft", bufs=1) as soft,
            tc.tile_pool(name="ps_a", bufs=2, space="PSUM") as ps_a,
            tc.tile_pool(name="ps_b", bufs=1, space="PSUM") as ps_b,
            tc.tile_pool(name="ps_c", bufs=1, space="PSUM") as ps_c,
            tc.tile_pool(name="ps_t", bufs=1, space="PSUM") as ps_t,
            tc.tile_pool(name="ps_at", bufs=2, space="PSUM") as ps_at,
            tc.tile_pool(name="ps_l", bufs=1, space="PSUM") as ps_l,
        ):
            # ---- load constants ----
            def cload(dram, shape, dtype, tag):
                t_ = consts.tile(shape, dtype, tag=tag)
                nc.sync.dma_start(out=t_[:], in_=dram[:])
                return t_

            qT_sb = cload(d_intT, [H, B], bf16, "qT")
            w_sb = {k: cload(v, list(v.shape), bf16, "w_" + k) for k, v in d_w.items()}
            w1k_sb = cload(d_w1k, [H, HID1], bf16, "w1k")
            w1p_sb = cload(d_w1p, [H, HID1], bf16, "w1p")
            w1q_sb = cload(d_w1q, [H, HID1], bf16, "w1q")
            w2_sb = cload(d_w2, [HID1, HID2], bf16, "w2")
            wf_sb = cload(d_wf, [HID2, 1], bf16, "wf")
            identrep_sb = cload(d_identrep, [B, 512], bf16, "identrep")
            maskadd_sb = cload(d_maskadd, [B, T], f32, "maskadd")
            mmask_sb = cload(d_mmask, [B, T], f32, "mmask")

            ident_f32 = consts.tile([128, 128], f32, tag="ident")
            make_identity(nc, ident_f32)

            h0_f32 = consts.tile([B, H], f32, tag="h0")
            nc.vector.memset(h0_f32[:], 0.0)
            hT0_bf = consts.tile([H, B], bf16, tag="hT0")
            nc.vector.memset(hT0_bf[:], 0.0)

            # keys chunks
            keys_ch = []
            for ci in range(NCH):
                kt = keysp.tile([H, TC * B], bf16)
                nc.sync.dma_start(out=kt[:], in_=d_keysT[:, ci * TC * B:(ci + 1) * TC * B])
                keys_ch.append(kt)

            # pre1 = (W1a+W1c) @ q  in [B, HID1], cast bf16
            pre1_ps = ps_at.tile([B, HID1], f32, tag="at")
            nc.tensor.matmul(pre1_ps[:], qT_sb[:], w1q_sb[:], start=True, stop=True)
            pre1_bf = consts.tile([B, HID1], bf16, tag="pre1")
            nc.scalar.copy(pre1_bf[:], pre1_ps[:])

            logits_ps = ps_l.tile([B, T], f32)

            int_ch = []

            def gru_step(t, h_prev, hT_prev, x_src, x_sl, wpfx, scal_col, out_hT):
                """One GRU/AUGRU step. scal_col: [B,1] column fused into update.
                out_hT: destination AP for transposed bf16 new state."""
                psA = ps_a.tile([B, 256], f32)
                psB = ps_b.tile([B, 256], f32)
                # gi parts (x-stationary) first: schedulable ahead of h
                nc.tensor.matmul(psA[:], x_src[:, x_sl], w_sb[wpfx + "_wih_rz"][:], start=True, stop=False)
                nc.tensor.matmul(psB[:, 128:256], x_src[:, x_sl], w_sb[wpfx + "_wih_n"][:], start=True, stop=True)
                # gh parts (h-stationary)
                nc.tensor.matmul(psA[:], hT_prev[:], w_sb[wpfx + "_whh_rz"][:], start=False, stop=True)
                nc.tensor.matmul(psB[:, 0:128], hT_prev[:], w_sb[wpfx + "_whh_n"][:], start=True, stop=True)

                rz = gatep.tile([B, 256], f32)
                nc.scalar.activation(rz[:], psA[:], AF.Sigmoid)
                t1 = small.tile([B, 128], f32)
                nc.vector.tensor_tensor(t1[:], rz[:, 0:128], psB[:, 0:128], OP.mult)
                psC = ps_c.tile([B, 128], f32)
                nc.vector.tensor_tensor(psC[:], t1[:], psB[:, 128:256], OP.add)
                n_sb = small.tile([B, 128], f32)
                nc.scalar.activation(n_sb[:], psC[:], AF.Tanh)
                d_sb = small.tile([B, 128], f32)
                nc.gpsimd.tensor_tensor(d_sb[:], n_sb[:], h_prev[:], OP.subtract)
                e_sb = small.tile([B, 128], f32)
                nc.vector.scalar_tensor_tensor(e_sb[:], rz[:, 128:256], scal_col, d_sb[:], OP.mult, OP.mult)
                h_new = state.tile([B, H], f32)
                nc.vector.tensor_tensor(h_new[:], h_prev[:], e_sb[:], OP.add)
                psT = ps_t.tile([H, B], f32)
                nc.tensor.transpose(psT[:], h_new[:], ident_f32[:])
                nc.scalar.copy(out_hT, psT[:])
                return h_new

            # ================= Phase E: interest-extractor GRU =================
            h_prev, hT_prev = h0_f32, hT0_bf
            for ci in range(NCH):
                ic = intp.tile([H, TC * B], bf16)
                int_ch.append(ic)
                qk = qkp.tile([H, TC * B], bf16)
                for j in range(TC):
                    t = ci * TC + j
                    sl = slice(j * B, (j + 1) * B)
                    h_prev = gru_step(
                        t, h_prev, hT_prev, keys_ch[ci], sl, "e",
                        mmask_sb[:, t:t + 1], ic[:, sl],
                    )
                    hT_prev = ic[:, sl]
                    # q*k for attention (bf16)
                    nc.gpsimd.tensor_tensor(qk[:, sl], ic[:, sl], qT_sb[:], OP.mult)
                # ---- attention MLP for this chunk ----
                h1 = attn_sb.tile([HID1, TC * B], bf16)
                h2 = attn_sb.tile([HID2, TC * B], bf16)
                for hf in range(2):
                    fsl = slice(hf * 512, (hf + 1) * 512)
                    h1ps = ps_at.tile([HID1, 512], f32, tag="at")
                    nc.tensor.matmul(h1ps[:], w1k_sb[:], ic[:, fsl], start=True, stop=False)
                    nc.tensor.matmul(h1ps[:], w1p_sb[:], qk[:, fsl], start=False, stop=False)
                    nc.tensor.matmul(h1ps[:], pre1_bf[:], identrep_sb[:], start=False, stop=True)
                    nc.scalar.activation(h1[:, fsl], h1ps[:], AF.Sigmoid)
                    h2ps = ps_at.tile([HID2, 512], f32, tag="at")
                    nc.tensor.matmul(h2ps[:], w2_sb[:], h1[:, fsl], start=True, stop=True)
                    nc.scalar.activation(h2[:, fsl], h2ps[:], AF.Sigmoid)
                for j in range(TC):
                    t = ci * TC + j
                    nc.tensor.matmul(
                        logits_ps[:, t:t + 1], h2[:, j * B:(j + 1) * B], wf_sb[:],
                        start=True, stop=True,
                    )

            # ================= softmax =================
            lm = soft.tile([B, T], f32)
            nc.vector.tensor_tensor(lm[:], logits_ps[:], maskadd_sb[:], OP.add)
            e_sm = soft.tile([B, T], f32)
            z_sm = soft.tile([B, 1], f32)
            nc.scalar.activation(e_sm[:], lm[:], AF.Exp, accum_out=z_sm[:])
            rz_sm = soft.tile([B, 1], f32)
            nc.vector.reciprocal(rz_sm[:], z_sm[:])
            att = soft.tile([B, T], f32)
            nc.vector.tensor_scalar(att[:], e_sm[:], rz_sm[:, 0:1], None, OP.mult)

            # ================= Phase A: AUGRU =================
            g_prev, gT_prev = h0_f32, hT0_bf
            for t in range(T):
                ci, j = divmod(t, TC)
                sl = slice(j * B, (j + 1) * B)
                gT_new = gatep.tile([H, B], bf16)
                g_prev = gru_step(
                    t, g_prev, gT_prev, int_ch[ci], sl, "a",
                    att[:, t:t + 1], gT_new[:],
                )
                gT_prev = gT_new

            nc.sync.dma_start(out=d_out[:], in_=g_prev[:])

    nc.compile()
    return nc


def _get_program():
    global _PROG
    if _PROG is None:
        _PROG = _build_program()
    return _PROG


def _bf(x):
    return np.ascontiguousarray(x.astype(ml_dtypes.bfloat16))


def kernel(**inputs):
    global LAST_EXEC_NS
    import time as _time
    _tk0 = _time.time()
    from concourse.bass_utils import run_bass_kernel_spmd

    nc = _get_program()
    globals()['BUILD_S'] = _time.time() - _tk0
    _tp0 = _time.time()

    query = np.asarray(inputs["query"], np.float32)
    keys = np.asarray(inputs["keys"], np.float32)
    keys_length = np.asarray(inputs["keys_length"]).astype(np.int64)
    Wih_e = np.asarray(inputs["Wih_e"], np.float32)
    Whh_e = np.asarray(inputs["Whh_e"], np.float32)
    Wih_a = np.asarray(inputs["Wih_a"], np.float32)
    Whh_a = np.asarray(inputs["Whh_a"], np.float32)
    W1 = np.asarray(inputs["W1"], np.float32)
    W2 = np.asarray(inputs["W2"], np.float32)
    Wf = np.asarray(inputs["Wf"], np.float32)
    bf_ = np.asarray(inputs["bf"], np.float32)

    # weight prep (shared across cores)
    def gru_w(Wih, Whh, negate_z):
        zsgn = -1.0 if negate_z else 1.0
        whh_rz = np.concatenate([Whh[0:128].T, zsgn * Whh[128:256].T], axis=1)
        wih_rz = np.concatenate([Wih[0:128].T, zsgn * Wih[128:256].T], axis=1)
        return {
            "whh_rz": _bf(whh_rz), "whh_n": _bf(Whh[256:384].T),
            "wih_rz": _bf(wih_rz), "wih_n": _bf(Wih[256:384].T),
        }

    we = gru_w(Wih_e, Whh_e, True)
    wa = gru_w(Wih_a, Whh_a, False)
    w1q = _bf((W1[:, 0:128] + W1[:, 256:384]).T)
    w1k = _bf((W1[:, 128:256] - W1[:, 256:384]).T)
    w1p = _bf(W1[:, 384:512].T)
    w2 = _bf(W2.T)
    wf_s = _bf((Wf[0] / np.sqrt(np.float32(H))).reshape(HID2, 1))
    identrep = _bf(np.tile(np.eye(B, dtype=np.float32), (1, 4)))

    shared = {
        "e_whh_rz": we["whh_rz"], "e_whh_n": we["whh_n"],
        "e_wih_rz": we["wih_rz"], "e_wih_n": we["wih_n"],
        "a_whh_rz": wa["whh_rz"], "a_whh_n": wa["whh_n"],
        "a_wih_rz": wa["wih_rz"], "a_wih_n": wa["wih_n"],
        "w1k": w1k, "w1p": w1p, "w1q": w1q, "w2": w2, "wf": wf_s,
        "identrep": identrep,
    }

    tvec = np.arange(T)
    bf_scaled = np.float32(bf_[0] / np.sqrt(np.float32(H)))

    in_maps = []
    for c in range(NCORES):
        rs = slice(c * B, (c + 1) * B)
        kl = keys_length[rs]
        valid = tvec[None, :] < kl[:, None]  # [B, T]
        maskadd = np.where(valid, bf_scaled, np.float32(-30000.0)).astype(np.float32)
        mmask = valid.astype(np.float32)
        keysT = _bf(keys[rs].transpose(2, 1, 0).reshape(H, T * B))
        qT = _bf(query[rs].T)
        m = dict(shared)
        m.update({"keysT": keysT, "qT": qT, "maskadd": maskadd, "mmask": mmask})
        in_maps.append(m)

    globals()['PREP_S'] = _time.time() - _tp0
    trace = os.environ.get("KERNEL_TRACE", "0") not in ("", "0")
    _t0 = _time.time()
    res = run_bass_kernel_spmd(nc, in_maps, core_ids=list(range(NCORES)), trace=trace)
    globals()['LAST_RUN_S'] = _time.time() - _t0
    LAST_EXEC_NS = res.exec_time_ns

    out = np.concatenate([res.results[c]["out"] for c in range(NCORES)], axis=0)
    return out.astype(np.float32)

